# Initial kernel scaffold
#
"""Causal no-head self-attention with RoPE on 8 Trainium2 NeuronCores.

Sharding: 8 cores = 4 batches x 2 balanced causal query-sets (zigzag):
  core (b, 0): query blocks [0:512) and [1536:2048)   (kt-structure 8, 16)
  core (b, 1): query blocks [512:1024) and [1024:1536) (kt-structure 8, 16)
All cores run ONE identical Bass program; per-core differences (which
queries, causal masks, RoPE angles) are carried in the input data.

Device layouts (transposed, channel-on-partition):
  QT/KT: [d_k, seq] with d_k de-interleaved (even feats rows 0:512, odd
  512:1024) so RoPE is a contiguous-partition-block rotation. The same
  permutation is applied to Wq/Wk rows on host (scores are invariant).
  V: natural [seq, d_v]. All matmuls in float32r (full PE rate at N>=256).
"""

import numpy as np
import sys

for _p in ("/opt/trn_rl_repo",):
    if _p not in sys.path:
        sys.path.insert(0, _p)

import concourse.bass as bass
import concourse.mybir as mybir
from concourse.tile import TileContext
from concourse.bass_utils import run_bass_kernel_spmd

B, S, D = 4, 2048, 1024
THETA = 10000.0
P = 128
NT = D // P          # 8 partition-tiles over a 1024 dim
QB = 512             # query block width (2 blocks per core)
NKT_A, NKT_B = 8, 16  # kt visits for q-block A / B (uniform structure)
F32 = mybir.dt.float32
F32R = mybir.dt.float32r
BF16 = mybir.dt.bfloat16
SCALE = 1.0 / 32.0   # 1/sqrt(D)


def _build_program():
    nc = bass.Bass(arch="trainium2")
    inp = {}
    def din(name, shape, dt):
        inp[name] = nc.dram_tensor(name, shape, dt, kind="ExternalInput")
    din("xT", [D, S], F32R)
    din("xTq", [D, 2 * QB], F32R)
    din("WqT", [D, D], F32R)
    din("WkT", [D, D], F32R)
    din("WvT", [D, D], F32R)
    din("WoT", [D, D], F32R)
    din("cosK", [D // 2, S], F32)
    din("sinK", [D // 2, S], F32)
    din("cosQ", [D // 2, 2 * QB], F32)
    din("sinQ", [D // 2, 2 * QB], F32)
    din("maskA", [P, NKT_A, QB], BF16)
    din("maskB", [P, NKT_B, QB], BF16)
    outT = nc.dram_tensor("outT", [P, NT, 2 * QB], F32, kind="ExternalOutput")

    xT_r = inp["xT"].rearrange("(t p) s -> p t s", p=P)
    xTq_r = inp["xTq"].rearrange("(t p) s -> p t s", p=P)
    WqT_r = inp["WqT"].rearrange("(t p) o -> p t o", p=P)
    WkT_r = inp["WkT"].rearrange("(t p) o -> p t o", p=P)
    WvT_r = inp["WvT"].rearrange("(t p) o -> p t o", p=P)
    WoT_r = inp["WoT"].rearrange("(t p) o -> p t o", p=P)
    cosK_r = inp["cosK"].rearrange("(t p) s -> p t s", p=P)
    sinK_r = inp["sinK"].rearrange("(t p) s -> p t s", p=P)
    cosQ_r = inp["cosQ"].rearrange("(t p) s -> p t s", p=P)
    sinQ_r = inp["sinQ"].rearrange("(t p) s -> p t s", p=P)

    with TileContext(nc) as tc:
        with (
            tc.tile_pool(name="dram", bufs=1, space="DRAM") as dpool,
            tc.tile_pool(name="wres", bufs=1) as wres,       # resident weight (32KB)
            tc.tile_pool(name="wres2", bufs=1) as wres2,     # second resident weight
            tc.tile_pool(name="xb", bufs=2) as xbp,          # x blocks
            tc.tile_pool(name="raw", bufs=2) as rawp,        # pre-RoPE proj blocks
            tc.tile_pool(name="rot", bufs=2) as rotp,        # post-RoPE blocks
            tc.tile_pool(name="cs", bufs=2) as csp,          # cos/sin tiles
            tc.tile_pool(name="tmp", bufs=2) as tmpp,        # RoPE scratch
            tc.tile_pool(name="vb", bufs=2) as vbp,          # V spill blocks
            tc.tile_pool(name="qt", bufs=1) as qtp,          # resident QT
            tc.tile_pool(name="pt", bufs=1) as ptp,          # exp(scores)
            tc.tile_pool(name="at", bufs=1) as atp,          # attnT
            tc.tile_pool(name="kts", bufs=3) as ktsp,        # streamed KT tiles
            tc.tile_pool(name="vts", bufs=3) as vtsp,        # streamed V tiles
            tc.tile_pool(name="wot", bufs=4) as wotp,        # streamed Wo tiles
            tc.tile_pool(name="msk", bufs=1) as mskp,
            tc.tile_pool(name="small", bufs=2) as smp,
            tc.tile_pool(name="outb", bufs=2) as outp,
            tc.tile_pool(name="psA", bufs=4, space="PSUM") as psA,
            tc.tile_pool(name="psB", bufs=2, space="PSUM") as psB,
            tc.tile_pool(name="psS", bufs=1, space="PSUM") as psS,
        ):
            KTd = dpool.tile([P, NT, S], F32R)       # RoPE'd K^T spill
            Vd = dpool.tile([P, S // P, D], F32R)    # V spill [p, kt, dv]

            ones_col = smp.tile([P, 1], F32R, tag="onescol")
            nc.vector.memset(ones_col[:], 1.0)
            ones_row = smp.tile([1, P], F32R, tag="onesrow")
            nc.vector.memset(ones_row[:], 1.0)

            def rope_block(dst, src, cos_t, sin_t, width):
                # dst/src: [P, NT, width]; rows 0:NT/2 = even feats, NT/2:NT = odd
                h = NT // 2
                e, o = src[:, 0:h, :width], src[:, h:NT, :width]
                c, s = cos_t[:, :, :width], sin_t[:, :, :width]
                t1 = tmpp.tile([P, h, QB], F32, tag="ropetmp")[:, :, :width]
                nc.vector.tensor_mul(out=dst[:, 0:h, :width], in0=e, in1=c)
                nc.vector.tensor_mul(out=t1, in0=o, in1=s)
                nc.vector.tensor_tensor(dst[:, 0:h, :width], dst[:, 0:h, :width],
                                        t1, mybir.AluOpType.subtract)
                t2 = tmpp.tile([P, h, QB], F32, tag="ropetmp")[:, :, :width]
                nc.vector.tensor_mul(out=dst[:, h:NT, :width], in0=o, in1=c)
                nc.vector.tensor_mul(out=t2, in0=e, in1=s)
                nc.vector.tensor_tensor(dst[:, h:NT, :width], dst[:, h:NT, :width],
                                        t2, mybir.AluOpType.add)

            # ---------- Phase 0: K^T and V projection (fused over seq blocks) ----
            WkRes = wres.tile([P, NT, D], F32R, tag="w")
            for t in range(NT):
                nc.sync.dma_start(WkRes[:, t, :], WkT_r[:, t, :])
            WvRes = wres2.tile([P, NT, D], F32R, tag="w2")
            for t in range(NT):
                nc.sync.dma_start(WvRes[:, t, :], WvT_r[:, t, :])

            for sb in range(S // QB):           # 4 seq blocks of 512
                sl = slice(sb * QB, (sb + 1) * QB)
                xb = xbp.tile([P, NT, QB], F32R, tag="xb")
                nc.sync.dma_start(xb[:], xT_r[:, :, sl])
                # K^T block: out rows t_out, cols = keys in this block
                kraw = rawp.tile([P, NT, QB], F32, tag="raw")
                for t_out in range(NT):
                    ps = psB.tile([P, QB], F32, tag="psB")
                    for dt_ in range(NT):
                        nc.tensor.matmul(ps[:], WkRes[:, dt_, t_out * P:(t_out + 1) * P],
                                         xb[:, dt_, :], start=(dt_ == 0), stop=(dt_ == NT - 1))
                    nc.scalar.copy(kraw[:, t_out, :], ps[:])
                ck = csp.tile([P, NT // 2, QB], F32, tag="cs_c")
                nc.sync.dma_start(ck[:], cosK_r[:, :, sl])
                sk = csp.tile([P, NT // 2, QB], F32, tag="cs_s")
                nc.sync.dma_start(sk[:], sinK_r[:, :, sl])
                krot = rotp.tile([P, NT, QB], F32R, tag="rot")
                rope_block(krot, kraw, ck, sk, QB)
                nc.sync.dma_start(KTd[:, :, sl], krot[:])
                # V rows for this block: out[seq-chunk, dv]
                vb = vbp.tile([P, QB // P, D], F32R, tag="vb")
                for dvb in range(2):
                    for sk_ in range(QB // P):
                        ps = psA.tile([P, QB], F32, tag="psA")
                        for dt_ in range(NT):
                            nc.tensor.matmul(ps[:], xb[:, dt_, sk_ * P:(sk_ + 1) * P],
                                             WvRes[:, dt_, dvb * QB:(dvb + 1) * QB],
                                             start=(dt_ == 0), stop=(dt_ == NT - 1))
                        nc.scalar.copy(vb[:, sk_, dvb * QB:(dvb + 1) * QB], ps[:])
                nc.sync.dma_start(Vd[:, sb * (QB // P):(sb + 1) * (QB // P), :], vb[:])

            # ---------- Phase 1: Q^T projection + RoPE (both q blocks) -----------
            WqRes = wres.tile([P, NT, D], F32R, tag="w")
            for t in range(NT):
                nc.sync.dma_start(WqRes[:, t, :], WqT_r[:, t, :])
            QT = qtp.tile([P, NT, 2 * QB], F32R, tag="qt")
            for qb in range(2):
                sl = slice(qb * QB, (qb + 1) * QB)
                xq = xbp.tile([P, NT, QB], F32R, tag="xb")
                nc.sync.dma_start(xq[:], xTq_r[:, :, sl])
                qraw = rawp.tile([P, NT, QB], F32, tag="raw")
                for t_out in range(NT):
                    ps = psB.tile([P, QB], F32, tag="psB")
                    for dt_ in range(NT):
                        nc.tensor.matmul(ps[:], WqRes[:, dt_, t_out * P:(t_out + 1) * P],
                                         xq[:, dt_, :], start=(dt_ == 0), stop=(dt_ == NT - 1))
                    nc.scalar.copy(qraw[:, t_out, :], ps[:])
                cq = csp.tile([P, NT // 2, QB], F32, tag="cs_c")
                nc.sync.dma_start(cq[:], cosQ_r[:, :, sl])
                sq = csp.tile([P, NT // 2, QB], F32, tag="cs_s")
                nc.sync.dma_start(sq[:], sinQ_r[:, :, sl])
                qrot = rotp.tile([P, NT, QB], F32R, tag="rot")
                rope_block(qrot, qraw, cq, sq, QB)
                nc.vector.tensor_copy(QT[:, :, sl], qrot[:])

            # ---------- Phase 2: attention + output projection per q block ------
            maskA_t = mskp.tile([P, NKT_A, QB], BF16, tag="mA")
            nc.sync.dma_start(maskA_t[:], inp["maskA"][:])
            maskB_t = mskp.tile([P, NKT_B, QB], BF16, tag="mB")
            nc.sync.dma_start(maskB_t[:], inp["maskB"][:])

            WoRes = wres.tile([P, NT, D], F32R, tag="w")
            for t in range(NT):
                nc.sync.dma_start(WoRes[:, t, :], WoT_r[:, t, :])

            for qb, nkt, mask_t in ((0, NKT_A, maskA_t), (1, NKT_B, maskB_t)):
                sl = slice(qb * QB, (qb + 1) * QB)
                PT = ptp.tile([P, NKT_B, QB], F32R, tag="pt")
                sums = psS.tile([1, QB], F32, tag="psS")
                for kt in range(nkt):
                    ktile = ktsp.tile([P, NT, P], F32R, tag="kts")
                    nc.sync.dma_start(ktile[:], KTd[:, :, kt * P:(kt + 1) * P])
                    ps = psB.tile([P, QB], F32, tag="psB")
                    for dt_ in range(NT):
                        nc.tensor.matmul(ps[:], ktile[:, dt_, :], QT[:, dt_, sl],
                                         start=(dt_ == 0), stop=(dt_ == NT - 1))
                    nc.scalar.activation(PT[:, kt, :], ps[:],
                                         mybir.ActivationFunctionType.Exp, scale=SCALE)
                    nc.vector.tensor_mul(out=PT[:, kt, :], in0=PT[:, kt, :],
                                         in1=mask_t[:, kt, :])
                    nc.tensor.matmul(sums[:], ones_col[:], PT[:, kt, :],
                                     start=(kt == 0), stop=(kt == nkt - 1))
                recip = smp.tile([1, QB], F32, tag="recip")
                nc.vector.reciprocal(recip[:], sums[:])
                bc_ps = psB.tile([P, QB], F32, tag="psB")
                nc.tensor.matmul(bc_ps[:], ones_row[:], recip[:], start=True, stop=True)
                bc = smp.tile([P, QB], F32, tag="bc")
                nc.scalar.copy(bc[:], bc_ps[:])

                attnT = atp.tile([P, NT, QB], F32R, tag="at")
                for wave in range(2):
                    for dvc in range(wave * 4, wave * 4 + 4):
                        psv = psA.tile([P, QB], F32, tag="psA")
                        for kt in range(nkt):
                            vtile = vtsp.tile([P, D], F32R, tag="vts")
                            if dvc == wave * 4:
                                nc.sync.dma_start(vtile[:], Vd[:, kt, :])
                                vtsp._cache = getattr(vtsp, "_cache", {})
                            nc.tensor.matmul(psv[:], vtile[:, dvc * P:(dvc + 1) * P],
                                             PT[:, kt, :], start=(kt == 0), stop=(kt == nkt - 1))
                        nc.vector.tensor_mul(out=attnT[:, dvc, :], in0=psv[:], in1=bc[:])

                ob = outp.tile([P, NT, QB], F32, tag="outb")
                for oc in range(NT):
                    ps = psB.tile([P, QB], F32, tag="psB")
                    for dt_ in range(NT):
                        nc.tensor.matmul(ps[:], WoRes[:, dt_, oc * P:(oc + 1) * P],
                                         attnT[:, dt_, :], start=(dt_ == 0), stop=(dt_ == NT - 1))
                    nc.scalar.copy(ob[:, oc, :], ps[:])
                nc.sync.dma_start(outT.rearrange("p t s -> p t s")[:, :, sl], ob[:])

    return nc


def _host_inputs(x, Wq, Wk, Wv, Wo, token_positions):
    perm = np.concatenate([np.arange(0, D, 2), np.arange(1, D, 2)])
    WqTp = np.ascontiguousarray(Wq[perm].T.astype(np.float32))
    WkTp = np.ascontiguousarray(Wk[perm].T.astype(np.float32))
    WvT = np.ascontiguousarray(Wv.T.astype(np.float32))
    WoT = np.ascontiguousarray(Wo.T.astype(np.float32))
    inv_freq = (1.0 / (np.float32(THETA) **
                       (np.arange(0, D, 2, dtype=np.float32) / np.float32(D))))
    inv_freq = inv_freq.astype(np.float32)

    in_maps, metas = [], []
    for b in range(B):
        xT = np.ascontiguousarray(x[b].T.astype(np.float32))   # [D, S]
        pos = token_positions[b].astype(np.float32)
        ang = (pos[None, :] * inv_freq[:, None]).astype(np.float32)  # [D/2, S]
        cosF = np.cos(ang).astype(np.float32)
        sinF = np.sin(ang).astype(np.float32)
        for h in range(2):
            if h == 0:
                qcols = np.r_[0:QB, 3 * QB:4 * QB]
                q0s = (0, 3 * QB)          # global start of q-block A, B
            else:
                qcols = np.r_[QB:2 * QB, 2 * QB:3 * QB]
                q0s = (QB, 2 * QB)
            xTq = np.ascontiguousarray(xT[:, qcols])
            cosQ = np.ascontiguousarray(cosF[:, qcols])
            sinQ = np.ascontiguousarray(sinF[:, qcols])
            masks = []
            for (q0, nkt) in zip(q0s, (NKT_A, NKT_B)):
                m = np.zeros((P, nkt, QB), dtype=np.float32)
                for kt in range(nkt):
                    kbase = kt * P
                    # valid iff (q0 + q) >= (kbase + k)
                    q_glob = q0 + np.arange(QB)
                    k_glob = kbase + np.arange(P)
                    m[:, kt, :] = (q_glob[None, :] >= k_glob[:, None])
                masks.append(m)
            import ml_dtypes
            maskA = masks[0].astype(ml_dtypes.bfloat16)
            maskB = masks[1].astype(ml_dtypes.bfloat16)
            in_maps.append({
                "xT": xT, "xTq": xTq,
                "WqT": WqTp, "WkT": WkTp, "WvT": WvT, "WoT": WoT,
                "cosK": cosF, "sinK": sinF, "cosQ": cosQ, "sinQ": sinQ,
                "maskA": maskA, "maskB": maskB,
            })
            metas.append((b, qcols))
    return in_maps, metas


_NC_CACHE = {}


def kernel(x, Wq, Wk, Wv, Wo, token_positions):
    x = np.asarray(x); token_positions = np.asarray(token_positions)
    if "nc" not in _NC_CACHE:
        _NC_CACHE["nc"] = _build_program()
    nc = _NC_CACHE["nc"]
    in_maps, metas = _host_inputs(np.asarray(x), np.asarray(Wq), np.asarray(Wk),
                                  np.asarray(Wv), np.asarray(Wo), token_positions)
    res = run_bass_kernel_spmd(nc, in_maps, core_ids=list(range(8)))
    out = np.empty((B, S, D), dtype=np.float32)
    for (b, qcols), r in zip(metas, res.results):
        oT = r["outT"]                       # [P, NT, 2*QB]
        o = np.transpose(oT, (2, 1, 0)).reshape(2 * QB, D)
        out[b, qcols, :] = o
    return out


# revision 16
# speedup vs baseline: 1.1181x; 1.1181x over previous
"""Causal no-head self-attention with RoPE on 8 Trainium2 NeuronCores.

Sharding: 8 cores = 4 batches x 2 balanced causal query-sets (zigzag):
  core (b, 0): query blocks [0:512) and [1536:2048)   (kt-structure 8, 16)
  core (b, 1): query blocks [512:1024) and [1024:1536) (kt-structure 8, 16)
All cores run ONE identical Bass program; per-core differences (which
queries, causal masks, RoPE angles) are carried in the input data.

Device layouts (transposed, channel-on-partition):
  QT/KT: [d_k, seq] with d_k de-interleaved (even feats rows 0:512, odd
  512:1024) so RoPE is a contiguous-partition-block rotation. The same
  permutation is applied to Wq/Wk rows on host (scores are invariant).
  V: natural [seq, d_v]. All matmuls in float32r (full PE rate at N>=256).
"""

import numpy as np
import sys

for _p in ("/opt/trn_rl_repo",):
    if _p not in sys.path:
        sys.path.insert(0, _p)

import concourse.bass as bass
import concourse.bacc as bacc
import concourse.mybir as mybir
from concourse.tile import TileContext
from concourse.bass_utils import run_bass_kernel_spmd

B, S, D = 4, 2048, 1024
THETA = 10000.0
P = 128
NT = D // P          # 8 partition-tiles over a 1024 dim
QB = 512             # query block width (2 blocks per core)
NKT_A, NKT_B = 8, 16  # kt visits for q-block A / B (uniform structure)
F32 = mybir.dt.float32
F32R = mybir.dt.float32r
BF16 = mybir.dt.bfloat16
SCALE = 1.0 / 32.0   # 1/sqrt(D)


def _build_program():
    nc = bacc.Bacc("TRN2", num_swdge_queues=4)
    inp = {}
    def din(name, shape, dt):
        inp[name] = nc.dram_tensor(name, shape, dt, kind="ExternalInput")
    din("xT", [D, S], F32R)
    din("xTq", [D, 2 * QB], F32R)
    din("WqT", [D, D], F32R)
    din("WkT", [D, D], F32R)
    din("WvT", [D, D], F32R)
    din("WoT", [D, D], F32R)
    din("cosK", [D // 2, S], F32)
    din("sinK", [D // 2, S], F32)
    din("cosQ", [D // 2, 2 * QB], F32)
    din("sinQ", [D // 2, 2 * QB], F32)
    din("ones_col", [P, 1], F32R)
    din("ones_row", [1, P], F32)
    din("maskA", [P, NKT_A, QB], BF16)
    din("maskB", [P, NKT_B, QB], BF16)
    outT = nc.dram_tensor("outT", [P, NT, 2 * QB], F32, kind="ExternalOutput")

    xT_r = inp["xT"].rearrange("(t p) s -> p t s", p=P)
    xTq_r = inp["xTq"].rearrange("(t p) s -> p t s", p=P)
    WqT_r = inp["WqT"].rearrange("(t p) o -> p t o", p=P)
    WkT_r = inp["WkT"].rearrange("(t p) o -> p t o", p=P)
    WvT_r = inp["WvT"].rearrange("(t p) o -> p t o", p=P)
    WoT_r = inp["WoT"].rearrange("(t p) o -> p t o", p=P)
    cosK_r = inp["cosK"].rearrange("(t p) s -> p t s", p=P)
    sinK_r = inp["sinK"].rearrange("(t p) s -> p t s", p=P)
    cosQ_r = inp["cosQ"].rearrange("(t p) s -> p t s", p=P)
    sinQ_r = inp["sinQ"].rearrange("(t p) s -> p t s", p=P)

    from contextlib import ExitStack
    with TileContext(nc) as tc:
        with ExitStack() as ctx:
            pool = lambda *a, **kw: ctx.enter_context(tc.tile_pool(*a, **kw))
            dpool = pool(name="dram", bufs=1, space="DRAM")
            wres = pool(name="wres", bufs=1)        # resident weight (32KB)
            smp = pool(name="small", bufs=2)
            psA = pool(name="psA", bufs=4, space="PSUM")
            psB = pool(name="psB", bufs=2, space="PSUM")
            psS = pool(name="psS", bufs=1, space="PSUM")

            KTd = dpool.tile([P, NT, S], F32R)       # RoPE'd K^T spill
            Vd = dpool.tile([P, S // P, D], F32R)    # V spill [p, kt, dv]

            ones_col = smp.tile([P, 1], F32R, tag="onescol")
            nc.sync.dma_start(ones_col[:], inp["ones_col"][:])
            ones_row = smp.tile([1, P], F32, tag="onesrow")
            nc.sync.dma_start(ones_row[:], inp["ones_row"][:])

            def rope_block(dst, src, cos_t, sin_t, tmpp):
                # dst/src: [P, NT, QB]; rows 0:NT/2 = even feats, NT/2:NT = odd
                h = NT // 2
                e, o = src[:, 0:h, :], src[:, h:NT, :]
                c, s = cos_t[:, :, :], sin_t[:, :, :]
                t1 = tmpp.tile([P, h, QB], F32, tag="ropetmp", name="t1")
                nc.vector.tensor_mul(out=dst[:, 0:h, :], in0=e, in1=c)
                nc.vector.tensor_mul(out=t1[:], in0=o, in1=s)
                nc.vector.tensor_tensor(dst[:, 0:h, :], dst[:, 0:h, :],
                                        t1[:], mybir.AluOpType.subtract)
                t2 = tmpp.tile([P, h, QB], F32, tag="ropetmp", name="t2")
                nc.vector.tensor_mul(out=dst[:, h:NT, :], in0=o, in1=c)
                nc.vector.tensor_mul(out=t2[:], in0=e, in1=s)
                nc.vector.tensor_tensor(dst[:, h:NT, :], dst[:, h:NT, :],
                                        t2[:], mybir.AluOpType.add)

            # ---------- Phase 0: K^T and V projection (fused over seq blocks) ----
            with ExitStack() as p0:
                pp = lambda *a, **kw: p0.enter_context(tc.tile_pool(*a, **kw))
                xbp = pp(name="xb0", bufs=2)
                rawp = pp(name="raw0", bufs=1)
                rotp = pp(name="rot0", bufs=1)
                csp = pp(name="cs0", bufs=1)
                tmpp = pp(name="tmp0", bufs=1)
                wres2 = pp(name="wres2", bufs=1)
                vbp = pp(name="vb", bufs=1)

                WkRes = wres.tile([P, NT, D], F32R, tag="w")
                for t in range(NT):
                    nc.sync.dma_start(WkRes[:, t, :], WkT_r[:, t, :])
                WvRes = wres2.tile([P, NT, D], F32R, tag="w2")
                for t in range(NT):
                    nc.sync.dma_start(WvRes[:, t, :], WvT_r[:, t, :])

                for sb in range(S // QB):           # 4 seq blocks of 512
                    sl = slice(sb * QB, (sb + 1) * QB)
                    xb = xbp.tile([P, NT, QB], F32R, tag="xb")
                    nc.sync.dma_start(xb[:], xT_r[:, :, sl])
                    # K^T block: out rows t_out, cols = keys in this block
                    kraw = rawp.tile([P, NT, QB], F32, tag="raw")
                    for t_out in range(NT):
                        ps = psB.tile([P, QB], F32, tag="psB")
                        for dt_ in range(NT):
                            nc.tensor.matmul(ps[:], WkRes[:, dt_, t_out * P:(t_out + 1) * P],
                                             xb[:, dt_, :], start=(dt_ == 0), stop=(dt_ == NT - 1))
                        nc.scalar.copy(kraw[:, t_out, :], ps[:])
                    ck = csp.tile([P, NT // 2, QB], F32, tag="cs_c")
                    nc.gpsimd.dma_start(ck[:], cosK_r[:, :, sl])
                    sk = csp.tile([P, NT // 2, QB], F32, tag="cs_s")
                    nc.gpsimd.dma_start(sk[:], sinK_r[:, :, sl])
                    krot = rotp.tile([P, NT, QB], F32R, tag="rot")
                    rope_block(krot, kraw, ck, sk, tmpp)
                    nc.sync.dma_start(KTd[:, :, sl], krot[:])
                    # V rows for this block: out[seq-chunk, dv]
                    vb = vbp.tile([P, QB // P, D], F32R, tag="vb")
                    for dvb in range(2):
                        for sk_ in range(QB // P):
                            ps = psA.tile([P, QB], F32, tag="psA")
                            for dt_ in range(NT):
                                nc.tensor.matmul(ps[:], xb[:, dt_, sk_ * P:(sk_ + 1) * P],
                                                 WvRes[:, dt_, dvb * QB:(dvb + 1) * QB],
                                                 start=(dt_ == 0), stop=(dt_ == NT - 1))
                            nc.scalar.copy(vb[:, sk_, dvb * QB:(dvb + 1) * QB], ps[:])
                    nc.sync.dma_start(Vd[:, sb * (QB // P):(sb + 1) * (QB // P), :], vb[:])

            # ---------- Phase 1: Q^T projection + RoPE (both q blocks) -----------
            qtp = pool(name="qt", bufs=1)           # resident Q^T (32KB)
            QT = qtp.tile([P, NT, 2 * QB], F32R, tag="qt")
            with ExitStack() as p1:
                pp = lambda *a, **kw: p1.enter_context(tc.tile_pool(*a, **kw))
                xbp = pp(name="xb1", bufs=2)
                rawp = pp(name="raw1", bufs=1)
                rotp = pp(name="rot1", bufs=1)
                csp = pp(name="cs1", bufs=1)
                tmpp = pp(name="tmp1", bufs=1)

                WqRes = wres.tile([P, NT, D], F32R, tag="w")
                for t in range(NT):
                    nc.sync.dma_start(WqRes[:, t, :], WqT_r[:, t, :])
                for qb in range(2):
                    sl = slice(qb * QB, (qb + 1) * QB)
                    xq = xbp.tile([P, NT, QB], F32R, tag="xb")
                    nc.sync.dma_start(xq[:], xTq_r[:, :, sl])
                    qraw = rawp.tile([P, NT, QB], F32, tag="raw")
                    for t_out in range(NT):
                        ps = psB.tile([P, QB], F32, tag="psB")
                        for dt_ in range(NT):
                            nc.tensor.matmul(ps[:], WqRes[:, dt_, t_out * P:(t_out + 1) * P],
                                             xq[:, dt_, :], start=(dt_ == 0), stop=(dt_ == NT - 1))
                        nc.scalar.copy(qraw[:, t_out, :], ps[:])
                    cq = csp.tile([P, NT // 2, QB], F32, tag="cs_c")
                    nc.gpsimd.dma_start(cq[:], cosQ_r[:, :, sl])
                    sq = csp.tile([P, NT // 2, QB], F32, tag="cs_s")
                    nc.gpsimd.dma_start(sq[:], sinQ_r[:, :, sl])
                    qrot = rotp.tile([P, NT, QB], F32R, tag="rot")
                    rope_block(qrot, qraw, cq, sq, tmpp)
                    nc.vector.tensor_copy(QT[:, :, sl], qrot[:])

            # ---------- Phase 2: attention + output projection per q block ------
            ptp = pool(name="pt", bufs=1)           # exp(scores)
            atp = pool(name="at", bufs=1)           # attnT
            ktsp = pool(name="kts", bufs=4)         # streamed KT tiles
            vtsp = pool(name="vts", bufs=6)         # streamed V tiles
            mskp = pool(name="msk", bufs=1)
            outp = pool(name="outb", bufs=1)

            maskA_t = mskp.tile([P, NKT_A, QB], BF16, tag="mA")
            nc.gpsimd.dma_start(maskA_t[:], inp["maskA"][:])
            maskB_t = mskp.tile([P, NKT_B, QB], BF16, tag="mB")
            nc.gpsimd.dma_start(maskB_t[:], inp["maskB"][:])

            WoRes = wres.tile([P, NT, D], F32R, tag="w")
            for t in range(NT):
                nc.sync.dma_start(WoRes[:, t, :], WoT_r[:, t, :])

            for qb, nkt, mask_t in ((0, NKT_A, maskA_t), (1, NKT_B, maskB_t)):
                sl = slice(qb * QB, (qb + 1) * QB)
                PT = ptp.tile([P, NKT_B, QB], F32R, tag="pt")
                sums = psS.tile([1, QB], F32, tag="psS")
                for kt in range(nkt):
                    ktile = ktsp.tile([P, NT, P], F32R, tag="kts")
                    nc.sync.dma_start(ktile[:], KTd[:, :, kt * P:(kt + 1) * P])
                    ps = psB.tile([P, QB], F32, tag="psB")
                    for dt_ in range(NT):
                        nc.tensor.matmul(ps[:], ktile[:, dt_, :], QT[:, dt_, sl],
                                         start=(dt_ == 0), stop=(dt_ == NT - 1))
                    nc.scalar.activation(PT[:, kt, :], ps[:],
                                         mybir.ActivationFunctionType.Exp, scale=SCALE)
                    nc.vector.tensor_mul(out=PT[:, kt, :], in0=PT[:, kt, :],
                                         in1=mask_t[:, kt, :])
                    nc.tensor.matmul(sums[:], ones_col[:], PT[:, kt, :],
                                     start=(kt == 0), stop=(kt == nkt - 1))
                recip = smp.tile([1, QB], F32, tag="recip")
                nc.vector.reciprocal(recip[:], sums[:])
                bc_ps = psB.tile([P, QB], F32, tag="psB")
                nc.tensor.matmul(bc_ps[:], ones_row[:], recip[:], start=True, stop=True)
                bc = smp.tile([P, QB], F32, tag="bc")
                nc.scalar.copy(bc[:], bc_ps[:])

                attnT = atp.tile([P, NT, QB], F32R, tag="at")
                for wave in range(2):
                    psvs = [psA.tile([P, QB], F32, tag="psA", name=f"psv{_j}") for _j in range(4)]
                    for kt in range(nkt):
                        vtile = vtsp.tile([P, QB], F32R, tag="vts")
                        nc.sync.dma_start(vtile[:], Vd[:, kt, wave * QB:(wave + 1) * QB])
                        for j in range(4):
                            nc.tensor.matmul(psvs[j][:], vtile[:, j * P:(j + 1) * P],
                                             PT[:, kt, :], start=(kt == 0), stop=(kt == nkt - 1))
                    for j in range(4):
                        nc.vector.tensor_mul(out=attnT[:, wave * 4 + j, :],
                                             in0=psvs[j][:], in1=bc[:])

                ob = outp.tile([P, NT, QB], F32, tag="outb")
                for oc in range(NT):
                    ps = psB.tile([P, QB], F32, tag="psB")
                    for dt_ in range(NT):
                        nc.tensor.matmul(ps[:], WoRes[:, dt_, oc * P:(oc + 1) * P],
                                         attnT[:, dt_, :], start=(dt_ == 0), stop=(dt_ == NT - 1))
                    nc.scalar.copy(ob[:, oc, :], ps[:])
                nc.sync.dma_start(outT[:, :, sl], ob[:])

    nc.finalize()
    return nc


def _host_inputs(x, Wq, Wk, Wv, Wo, token_positions):
    perm = np.concatenate([np.arange(0, D, 2), np.arange(1, D, 2)])
    WqTp = np.ascontiguousarray(Wq[perm].T.astype(np.float32))
    WkTp = np.ascontiguousarray(Wk[perm].T.astype(np.float32))
    WvT = np.ascontiguousarray(Wv.T.astype(np.float32))
    WoT = np.ascontiguousarray(Wo.T.astype(np.float32))
    inv_freq = (1.0 / (np.float32(THETA) **
                       (np.arange(0, D, 2, dtype=np.float32) / np.float32(D))))
    inv_freq = inv_freq.astype(np.float32)

    in_maps, metas = [], []
    for b in range(B):
        xT = np.ascontiguousarray(x[b].T.astype(np.float32))   # [D, S]
        pos = token_positions[b].astype(np.float32)
        ang = (pos[None, :] * inv_freq[:, None]).astype(np.float32)  # [D/2, S]
        cosF = np.cos(ang).astype(np.float32)
        sinF = np.sin(ang).astype(np.float32)
        for h in range(2):
            if h == 0:
                qcols = np.r_[0:QB, 3 * QB:4 * QB]
                q0s = (0, 3 * QB)          # global start of q-block A, B
            else:
                qcols = np.r_[QB:2 * QB, 2 * QB:3 * QB]
                q0s = (QB, 2 * QB)
            xTq = np.ascontiguousarray(xT[:, qcols])
            cosQ = np.ascontiguousarray(cosF[:, qcols])
            sinQ = np.ascontiguousarray(sinF[:, qcols])
            masks = []
            for (q0, nkt) in zip(q0s, (NKT_A, NKT_B)):
                m = np.zeros((P, nkt, QB), dtype=np.float32)
                for kt in range(nkt):
                    kbase = kt * P
                    # valid iff (q0 + q) >= (kbase + k)
                    q_glob = q0 + np.arange(QB)
                    k_glob = kbase + np.arange(P)
                    m[:, kt, :] = (q_glob[None, :] >= k_glob[:, None])
                masks.append(m)
            import ml_dtypes
            maskA = masks[0].astype(ml_dtypes.bfloat16)
            maskB = masks[1].astype(ml_dtypes.bfloat16)
            in_maps.append({
                "ones_col": np.ones((P, 1), np.float32),
                "ones_row": np.ones((1, P), np.float32),
                "xT": xT, "xTq": xTq,
                "WqT": WqTp, "WkT": WkTp, "WvT": WvT, "WoT": WoT,
                "cosK": cosF, "sinK": sinF, "cosQ": cosQ, "sinQ": sinQ,
                "maskA": maskA, "maskB": maskB,
            })
            metas.append((b, qcols))
    return in_maps, metas


_NC_CACHE = {}


def kernel(x, Wq, Wk, Wv, Wo, token_positions):
    x = np.asarray(x); token_positions = np.asarray(token_positions)
    if "nc" not in _NC_CACHE:
        _NC_CACHE["nc"] = _build_program()
    nc = _NC_CACHE["nc"]
    in_maps, metas = _host_inputs(np.asarray(x), np.asarray(Wq), np.asarray(Wk),
                                  np.asarray(Wv), np.asarray(Wo), token_positions)
    res = run_bass_kernel_spmd(nc, in_maps, core_ids=list(range(8)))
    out = np.empty((B, S, D), dtype=np.float32)
    for (b, qcols), r in zip(metas, res.results):
        oT = r["outT"]                       # [P, NT, 2*QB]
        o = np.transpose(oT, (2, 1, 0)).reshape(2 * QB, D)
        out[b, qcols, :] = o
    return out


# revision 20
# speedup vs baseline: 1.1814x; 1.0567x over previous
"""Causal no-head self-attention with RoPE on 8 Trainium2 NeuronCores.

Sharding: 8 cores = 4 batches x 2 balanced causal query-sets (zigzag):
  core (b, 0): query blocks [0:512) and [1536:2048)   (kt-structure 8, 16)
  core (b, 1): query blocks [512:1024) and [1024:1536) (kt-structure 8, 16)
All cores run ONE identical Bass program; per-core differences (which
queries, causal masks, RoPE angles) are carried in the input data.

Device layouts (transposed, channel-on-partition):
  QT/KT: [d_k, seq] with d_k de-interleaved (even feats rows 0:512, odd
  512:1024) so RoPE is a contiguous-partition-block rotation. The same
  permutation is applied to Wq/Wk rows on host (scores are invariant).
  V: natural [seq, d_v]. All matmuls in float32r (full PE rate at N>=256).
"""

import numpy as np
import sys

for _p in ("/opt/trn_rl_repo",):
    if _p not in sys.path:
        sys.path.insert(0, _p)

import concourse.bass as bass
import concourse.bacc as bacc
import concourse.mybir as mybir
from concourse.tile import TileContext
from concourse.bass_utils import run_bass_kernel_spmd

B, S, D = 4, 2048, 1024
THETA = 10000.0
P = 128
NT = D // P          # 8 partition-tiles over a 1024 dim
QB = 512             # query block width (2 blocks per core)
NKT_A, NKT_B = 8, 16  # kt visits for q-block A / B (uniform structure)
F32 = mybir.dt.float32
F32R = mybir.dt.float32r
BF16 = mybir.dt.bfloat16
SCALE = 1.0 / 32.0   # 1/sqrt(D)


def _build_program():
    nc = bacc.Bacc("TRN2", num_swdge_queues=4)
    inp = {}
    def din(name, shape, dt):
        inp[name] = nc.dram_tensor(name, shape, dt, kind="ExternalInput")
    din("xT", [D, S], F32R)
    din("xTq", [D, 2 * QB], F32R)
    din("WqT", [D, D], F32R)
    din("WkT", [D, D], F32R)
    din("WvT", [D, D], F32R)
    din("WoT", [D, D], F32R)
    din("cosK", [D // 2, S], F32)
    din("sinK", [D // 2, S], F32)
    din("cosQ", [D // 2, 2 * QB], F32)
    din("sinQ", [D // 2, 2 * QB], F32)
    din("ones_col", [P, 1], F32R)
    din("ones_row", [1, P], F32)
    din("maskA", [P, NKT_A, QB], BF16)
    din("maskB", [P, NKT_B, QB], BF16)
    outT = nc.dram_tensor("outT", [P, NT, 2 * QB], F32, kind="ExternalOutput")

    xT_r = inp["xT"].rearrange("(t p) s -> p t s", p=P)
    xTq_r = inp["xTq"].rearrange("(t p) s -> p t s", p=P)
    WqT_r = inp["WqT"].rearrange("(t p) o -> p t o", p=P)
    WkT_r = inp["WkT"].rearrange("(t p) o -> p t o", p=P)
    WvT_r = inp["WvT"].rearrange("(t p) o -> p t o", p=P)
    WoT_r = inp["WoT"].rearrange("(t p) o -> p t o", p=P)
    cosK_r = inp["cosK"].rearrange("(t p) s -> p t s", p=P)
    sinK_r = inp["sinK"].rearrange("(t p) s -> p t s", p=P)
    cosQ_r = inp["cosQ"].rearrange("(t p) s -> p t s", p=P)
    sinQ_r = inp["sinQ"].rearrange("(t p) s -> p t s", p=P)

    from contextlib import ExitStack
    with TileContext(nc) as tc:
        with ExitStack() as ctx:
            pool = lambda *a, **kw: ctx.enter_context(tc.tile_pool(*a, **kw))
            dpool = pool(name="dram", bufs=1, space="DRAM")
            wres = pool(name="wres", bufs=1)        # resident weight (32KB)
            smp = pool(name="small", bufs=1)
            psA = pool(name="psA", bufs=4, space="PSUM")
            psB = pool(name="psB", bufs=2, space="PSUM")
            psS = pool(name="psS", bufs=1, space="PSUM")

            # Per-seq-block spill tiles: a kt-tile read only waits on its
            # own block's spill write, so attention overlaps late KV phase.
            KTd = [dpool.tile([QB // P, P, NT, P], F32R, name=f"ktd{i}") for i in range(S // QB)]
            Vd = [dpool.tile([P, QB // P, D], F32R, name=f"vd{i}") for i in range(S // QB)]

            ones_col = smp.tile([P, 1], F32R, tag="onescol")
            nc.sync.dma_start(ones_col[:], inp["ones_col"][:])
            ones_row = smp.tile([1, P], F32, tag="onesrow")
            nc.sync.dma_start(ones_row[:], inp["ones_row"][:])

            def rope_block(dst, src, cos_t, sin_t, tmpp):
                # dst/src: [P, NT, QB]; rows 0:NT/2 = even feats, NT/2:NT = odd
                h = NT // 2
                e, o = src[:, 0:h, :], src[:, h:NT, :]
                c, s = cos_t[:, :, :], sin_t[:, :, :]
                t1 = tmpp.tile([P, h, QB], F32, tag="ropetmp", name="t1")
                nc.vector.tensor_mul(out=dst[:, 0:h, :], in0=e, in1=c)
                nc.vector.tensor_mul(out=t1[:], in0=o, in1=s)
                nc.vector.tensor_tensor(dst[:, 0:h, :], dst[:, 0:h, :],
                                        t1[:], mybir.AluOpType.subtract)
                t2 = tmpp.tile([P, h, QB], F32, tag="ropetmp", name="t2")
                nc.vector.tensor_mul(out=dst[:, h:NT, :], in0=o, in1=c)
                nc.vector.tensor_mul(out=t2[:], in0=e, in1=s)
                nc.vector.tensor_tensor(dst[:, h:NT, :], dst[:, h:NT, :],
                                        t2[:], mybir.AluOpType.add)

            # ---------- Phase 1: Q^T projection + RoPE (both q blocks) -----------
            qtp = pool(name="qt", bufs=1)           # resident Q^T (32KB)
            QT = qtp.tile([P, NT, 2 * QB], F32R, tag="qt")
            with ExitStack() as p1:
                pp = lambda *a, **kw: p1.enter_context(tc.tile_pool(*a, **kw))
                xbp = pp(name="xb1", bufs=2)
                rawp = pp(name="raw1", bufs=1)
                rotp = pp(name="rot1", bufs=1)
                csp = pp(name="cs1", bufs=1)
                tmpp = pp(name="tmp1", bufs=1)

                WqRes = wres.tile([P, NT, D], F32R, tag="w")
                for t in range(NT):
                    nc.sync.dma_start(WqRes[:, t, :], WqT_r[:, t, :])
                for qb in range(2):
                    sl = slice(qb * QB, (qb + 1) * QB)
                    xq = xbp.tile([P, NT, QB], F32R, tag="xb")
                    nc.sync.dma_start(xq[:], xTq_r[:, :, sl])
                    qraw = rawp.tile([P, NT, QB], F32, tag="raw")
                    for t_out in range(NT):
                        ps = psB.tile([P, QB], F32, tag="psB")
                        for dt_ in range(NT):
                            nc.tensor.matmul(ps[:], WqRes[:, dt_, t_out * P:(t_out + 1) * P],
                                             xq[:, dt_, :], start=(dt_ == 0), stop=(dt_ == NT - 1))
                        nc.scalar.copy(qraw[:, t_out, :], ps[:])
                    cq = csp.tile([P, NT // 2, QB], F32, tag="cs_c")
                    nc.gpsimd.dma_start(cq[:], cosQ_r[:, :, sl])
                    sq = csp.tile([P, NT // 2, QB], F32, tag="cs_s")
                    nc.gpsimd.dma_start(sq[:], sinQ_r[:, :, sl])
                    qrot = rotp.tile([P, NT, QB], F32R, tag="rot")
                    rope_block(qrot, qraw, cq, sq, tmpp)
                    nc.vector.tensor_copy(QT[:, :, sl], qrot[:])

            # ---------- Phase 0: K^T and V projection (fused over seq blocks) ----
            with ExitStack() as p0:
                pp = lambda *a, **kw: p0.enter_context(tc.tile_pool(*a, **kw))
                xbp = pp(name="xb0", bufs=2)
                rawp = pp(name="raw0", bufs=1)
                rotp = pp(name="rot0", bufs=1)
                csp = pp(name="cs0", bufs=1)
                tmpp = pp(name="tmp0", bufs=1)
                wres2 = pp(name="wres2", bufs=1)
                vbp = pp(name="vb", bufs=1)

                WkRes = wres.tile([P, NT, D], F32R, tag="w")
                for t in range(NT):
                    nc.sync.dma_start(WkRes[:, t, :], WkT_r[:, t, :])
                WvRes = wres2.tile([P, NT, D], F32R, tag="w2")
                for t in range(NT):
                    nc.sync.dma_start(WvRes[:, t, :], WvT_r[:, t, :])

                for sb in range(S // QB):           # 4 seq blocks of 512
                    sl = slice(sb * QB, (sb + 1) * QB)
                    xb = xbp.tile([P, NT, QB], F32R, tag="xb")
                    nc.sync.dma_start(xb[:], xT_r[:, :, sl])
                    # K^T block: out rows t_out, cols = keys in this block
                    kraw = rawp.tile([P, NT, QB], F32, tag="raw")
                    for t_out in range(NT):
                        ps = psB.tile([P, QB], F32, tag="psB")
                        for dt_ in range(NT):
                            nc.tensor.matmul(ps[:], WkRes[:, dt_, t_out * P:(t_out + 1) * P],
                                             xb[:, dt_, :], start=(dt_ == 0), stop=(dt_ == NT - 1))
                        nc.scalar.copy(kraw[:, t_out, :], ps[:])
                    ck = csp.tile([P, NT // 2, QB], F32, tag="cs_c")
                    nc.gpsimd.dma_start(ck[:], cosK_r[:, :, sl])
                    sk = csp.tile([P, NT // 2, QB], F32, tag="cs_s")
                    nc.gpsimd.dma_start(sk[:], sinK_r[:, :, sl])
                    krot = rotp.tile([P, NT, QB], F32R, tag="rot")
                    rope_block(krot, kraw, ck, sk, tmpp)
                    for ks in range(QB // P):
                        nc.sync.dma_start(KTd[sb][ks], krot[:, :, ks * P:(ks + 1) * P])
                    # V rows for this block: out[seq-chunk, dv]
                    vb = vbp.tile([P, QB // P, D], F32R, tag="vb")
                    for dvb in range(2):
                        for sk_ in range(QB // P):
                            ps = psA.tile([P, QB], F32, tag="psA")
                            for dt_ in range(NT):
                                nc.tensor.matmul(ps[:], xb[:, dt_, sk_ * P:(sk_ + 1) * P],
                                                 WvRes[:, dt_, dvb * QB:(dvb + 1) * QB],
                                                 start=(dt_ == 0), stop=(dt_ == NT - 1))
                            nc.scalar.copy(vb[:, sk_, dvb * QB:(dvb + 1) * QB], ps[:])
                    nc.sync.dma_start(Vd[sb][:], vb[:])

            # ---------- Phase 2: attention + output projection per q block ------
            ptp = pool(name="pt", bufs=1)           # exp(scores)
            atp = pool(name="at", bufs=1)           # attnT
            ktsp = pool(name="kts", bufs=4)         # streamed KT tiles
            vtsp = pool(name="vts", bufs=6)         # streamed V tiles
            mskp = pool(name="msk", bufs=1)
            outp = pool(name="outb", bufs=1)

            maskA_t = mskp.tile([P, NKT_A, QB], BF16, tag="mA")
            nc.gpsimd.dma_start(maskA_t[:], inp["maskA"][:])
            maskB_t = mskp.tile([P, NKT_B, QB], BF16, tag="mB")
            nc.gpsimd.dma_start(maskB_t[:], inp["maskB"][:])

            WoRes = wres.tile([P, NT, D], F32R, tag="w")
            for t in range(NT):
                nc.sync.dma_start(WoRes[:, t, :], WoT_r[:, t, :])

            for qb, nkt, mask_t in ((0, NKT_A, maskA_t), (1, NKT_B, maskB_t)):
                sl = slice(qb * QB, (qb + 1) * QB)
                PT = ptp.tile([P, NKT_B, QB], F32R, tag="pt")
                sums = psS.tile([1, QB], F32, tag="psS")
                for kt in range(nkt):
                    ktile = ktsp.tile([P, NT, P], F32R, tag="kts")
                    nc.sync.dma_start(ktile[:], KTd[kt // (QB // P)][kt % (QB // P)])
                    ps = psB.tile([P, QB], F32, tag="psB")
                    for dt_ in range(NT):
                        nc.tensor.matmul(ps[:], ktile[:, dt_, :], QT[:, dt_, sl],
                                         start=(dt_ == 0), stop=(dt_ == NT - 1))
                    nc.scalar.activation(PT[:, kt, :], ps[:],
                                         mybir.ActivationFunctionType.Exp, scale=SCALE)
                    nc.vector.tensor_mul(out=PT[:, kt, :], in0=PT[:, kt, :],
                                         in1=mask_t[:, kt, :])
                for kt in range(nkt):
                    nc.tensor.matmul(sums[:], ones_col[:], PT[:, kt, :],
                                     start=(kt == 0), stop=(kt == nkt - 1))
                recip = smp.tile([1, QB], F32, tag="recip")
                nc.vector.reciprocal(recip[:], sums[:])
                bc_ps = psB.tile([P, QB], F32, tag="psB")
                nc.tensor.matmul(bc_ps[:], ones_row[:], recip[:], start=True, stop=True)
                bc = smp.tile([P, QB], F32, tag="bc")
                nc.scalar.copy(bc[:], bc_ps[:])

                attnT = atp.tile([P, NT, QB], F32R, tag="at")
                for wave in range(2):
                    psvs = [psA.tile([P, QB], F32, tag="psA", name=f"psv{_j}") for _j in range(4)]
                    for kt in range(nkt):
                        vtile = vtsp.tile([P, QB], F32R, tag="vts")
                        nc.sync.dma_start(vtile[:], Vd[kt // (QB // P)][:, kt % (QB // P), wave * QB:(wave + 1) * QB])
                        for j in range(4):
                            nc.tensor.matmul(psvs[j][:], vtile[:, j * P:(j + 1) * P],
                                             PT[:, kt, :], start=(kt == 0), stop=(kt == nkt - 1))
                    for j in range(4):
                        nc.vector.tensor_mul(out=attnT[:, wave * 4 + j, :],
                                             in0=psvs[j][:], in1=bc[:])

                ob = outp.tile([P, NT, QB], F32, tag="outb")
                for oc in range(NT):
                    ps = psB.tile([P, QB], F32, tag="psB")
                    for dt_ in range(NT):
                        nc.tensor.matmul(ps[:], WoRes[:, dt_, oc * P:(oc + 1) * P],
                                         attnT[:, dt_, :], start=(dt_ == 0), stop=(dt_ == NT - 1))
                    nc.scalar.copy(ob[:, oc, :], ps[:])
                nc.sync.dma_start(outT[:, :, sl], ob[:])

    nc.finalize()
    return nc


def _host_inputs(x, Wq, Wk, Wv, Wo, token_positions):
    perm = np.concatenate([np.arange(0, D, 2), np.arange(1, D, 2)])
    WqTp = np.ascontiguousarray(Wq[perm].T.astype(np.float32))
    WkTp = np.ascontiguousarray(Wk[perm].T.astype(np.float32))
    WvT = np.ascontiguousarray(Wv.T.astype(np.float32))
    WoT = np.ascontiguousarray(Wo.T.astype(np.float32))
    inv_freq = (1.0 / (np.float32(THETA) **
                       (np.arange(0, D, 2, dtype=np.float32) / np.float32(D))))
    inv_freq = inv_freq.astype(np.float32)

    in_maps, metas = [], []
    for b in range(B):
        xT = np.ascontiguousarray(x[b].T.astype(np.float32))   # [D, S]
        pos = token_positions[b].astype(np.float32)
        ang = (pos[None, :] * inv_freq[:, None]).astype(np.float32)  # [D/2, S]
        cosF = np.cos(ang).astype(np.float32)
        sinF = np.sin(ang).astype(np.float32)
        for h in range(2):
            if h == 0:
                qcols = np.r_[0:QB, 3 * QB:4 * QB]
                q0s = (0, 3 * QB)          # global start of q-block A, B
            else:
                qcols = np.r_[QB:2 * QB, 2 * QB:3 * QB]
                q0s = (QB, 2 * QB)
            xTq = np.ascontiguousarray(xT[:, qcols])
            cosQ = np.ascontiguousarray(cosF[:, qcols])
            sinQ = np.ascontiguousarray(sinF[:, qcols])
            masks = []
            for (q0, nkt) in zip(q0s, (NKT_A, NKT_B)):
                m = np.zeros((P, nkt, QB), dtype=np.float32)
                for kt in range(nkt):
                    kbase = kt * P
                    # valid iff (q0 + q) >= (kbase + k)
                    q_glob = q0 + np.arange(QB)
                    k_glob = kbase + np.arange(P)
                    m[:, kt, :] = (q_glob[None, :] >= k_glob[:, None])
                masks.append(m)
            import ml_dtypes
            maskA = masks[0].astype(ml_dtypes.bfloat16)
            maskB = masks[1].astype(ml_dtypes.bfloat16)
            in_maps.append({
                "ones_col": np.ones((P, 1), np.float32),
                "ones_row": np.ones((1, P), np.float32),
                "xT": xT, "xTq": xTq,
                "WqT": WqTp, "WkT": WkTp, "WvT": WvT, "WoT": WoT,
                "cosK": cosF, "sinK": sinF, "cosQ": cosQ, "sinQ": sinQ,
                "maskA": maskA, "maskB": maskB,
            })
            metas.append((b, qcols))
    return in_maps, metas


_NC_CACHE = {}


def kernel(x, Wq, Wk, Wv, Wo, token_positions):
    x = np.asarray(x); token_positions = np.asarray(token_positions)
    if "nc" not in _NC_CACHE:
        _NC_CACHE["nc"] = _build_program()
    nc = _NC_CACHE["nc"]
    in_maps, metas = _host_inputs(np.asarray(x), np.asarray(Wq), np.asarray(Wk),
                                  np.asarray(Wv), np.asarray(Wo), token_positions)
    res = run_bass_kernel_spmd(nc, in_maps, core_ids=list(range(8)))
    out = np.empty((B, S, D), dtype=np.float32)
    for (b, qcols), r in zip(metas, res.results):
        oT = r["outT"]                       # [P, NT, 2*QB]
        o = np.transpose(oT, (2, 1, 0)).reshape(2 * QB, D)
        out[b, qcols, :] = o
    return out
